# revision 1
# baseline (speedup 1.0000x reference)
"""GAT layer kernel for Trainium2 (8 NeuronCores, Bass/Tile).

Problem: h = input_h @ W + bias; per-edge e = leakyrelu(wh1[row] + wh2[col]);
segment softmax over each destination row's 16 edges; out = segment_sum of
attn * h[col].

Strategy (see spec sharding hint): destination rows are sharded across the 8
cores (12500 rows each). Each core computes h/wh1/wh2 for its own rows into a
packed fp16 table, all-gathers the table, then gathers h[col]/wh2[col] per
edge with the GpSimd ucode `dma_gather` (int16 indices -> 4 nodes packed per
768B table row, idx = col>>2; the 1-of-4 sub-row selection is folded into the
PE aggregation as masked attention weights).

Layout notes:
- dma_gather places index position j at SBUF partition j%128, slot j//128
  ("edge-major"): dest d's 16 edges land in partitions 16*(d%8)..16*(d%8)+15
  of chunk d//8, so segment sums are 16-partition-span sums done on the PE
  with a constant 0/1 matrix, and per-dest scalars are expanded back with a
  constant [8,128] matrix.
- Host-side prep is limited to slicing/layout of the *index* input (col>>2 as
  replicated int16 wrap tiles, col&3 one-hot masks) plus constant matrices.
  All numerical work on h/W/a/bias happens on device.
"""

import sys
import types

import numpy as np

sys.path.insert(0, "/opt/trn_rl_repo")

# ---------------------------------------------------------------- constants
N = 100000
DEG = 16
E = N * DEG
IN_F = 128
OUT_F = 64
ALPHA = 0.2
EPS = 1e-12

NCORES = 8
NLOC = N // NCORES              # 12500 dest rows per core
ELOC = NLOC * DEG               # 200000 edges per core
P = 128
NBLK = (NLOC + P - 1) // P      # 98 blocks of 128 dests
EPAD = NBLK * P * DEG           # 200704 edge slots (padded)
LAST_VALID = NLOC - (NBLK - 1) * P  # 84 dests in last block

PACK = 4                        # nodes per table row
NPITCH = 96                     # fp16 elems per node slot (192B)
RSTRIDE = PACK * NPITCH         # 384 fp16 = 768B row stride
FETCH = (PACK - 1) * NPITCH + 66  # 354 fp16 = 708B fetched per edge
TROWS = N // PACK               # 25000 table rows
TROWS_LOC = NLOC // PACK        # 3125 local table rows
GIDX = 1024                     # indices per dma_gather (2 per block)
WH1PAD = NBLK * P               # 12544


def _install_ntff_shim():
    if "antenv.axon_hooks" in sys.modules:
        return
    try:
        from trn_agent_boot.trn_boot import _ntff_profile_via_ctypes

        hook = _ntff_profile_via_ctypes("/opt/axon/libaxon_pjrt.so")
    except Exception:
        hook = None
    mod = types.ModuleType("antenv.axon_hooks")
    mod.get_axon_ntff_profile_hook = lambda: hook
    mod.set_axon_ntff_profile_hook = lambda h: None
    sys.modules["antenv.axon_hooks"] = mod


def _install_dma_gather_patch():
    """Relax bass's elem_size%256 assert (ucode needs it only for transpose)."""
    import inspect
    import textwrap

    import concourse.bass as bass

    if getattr(bass.BassGpSimd.dma_gather, "_gat_patched", False):
        return
    src = textwrap.dedent(inspect.getsource(bass.BassGpSimd.dma_gather))
    old = """    assert (
        elem_size_bytes > 0 and elem_size_bytes % 256 == 0
    )  # transpose restriction"""
    new = """    assert elem_size_bytes > 0
    if transpose:
        assert elem_size_bytes % 256 == 0"""
    assert old in src, "dma_gather source changed; patch needs updating"
    src = src.replace(old, new)
    g = dict(bass.__dict__)
    exec(src, g)
    g["dma_gather"]._gat_patched = True
    bass.BassGpSimd.dma_gather = g["dma_gather"]


# ---------------------------------------------------------------- program
_PROGRAM_CACHE = {}


def build_program():
    _install_ntff_shim()
    _install_dma_gather_patch()
    import concourse.bacc as bacc
    import concourse.bass as bass
    import concourse.tile as tile
    from concourse import mybir

    f32 = mybir.dt.float32
    f16 = mybir.dt.float16
    i16 = mybir.dt.int16

    nc = bacc.Bacc(
        "TRN2",
        num_devices=NCORES,
        num_swdge_queues=4,
        detect_race_conditions=False,
    )

    # per-core external inputs
    inT = nc.dram_tensor("inT", [IN_F, NLOC], f32, kind="ExternalInput")
    W_in = nc.dram_tensor("W_in", [IN_F, OUT_F], f32, kind="ExternalInput")
    a2_in = nc.dram_tensor("a2_in", [OUT_F, 2], f32, kind="ExternalInput")
    bias_in = nc.dram_tensor("bias_in", [OUT_F], f32, kind="ExternalInput")
    idx16w = nc.dram_tensor("idx16w", [NBLK, 2, P, GIDX // 16], i16, kind="ExternalInput")
    oneh32 = nc.dram_tensor("oneh32", [NBLK, P, DEG * PACK], f32, kind="ExternalInput")
    soh16 = nc.dram_tensor("soh16", [NBLK, P, DEG * PACK * 8], f16, kind="ExternalInput")
    e8_in = nc.dram_tensor("e8_in", [8, P], f32, kind="ExternalInput")
    s8_in = nc.dram_tensor("s8_in", [P, 8], f32, kind="ExternalInput")
    id2_in = nc.dram_tensor("id2_in", [2, 2], f32, kind="ExternalInput")

    out_d = nc.dram_tensor("out_d", [NLOC, OUT_F], f32, kind="ExternalOutput")

    with tile.TileContext(nc) as tc:
        with tc.tile_pool(name="dram", bufs=1, space="DRAM") as dpool:
            h4_loc = dpool.tile([TROWS_LOC, RSTRIDE], f16)
            h4 = dpool.tile([TROWS, RSTRIDE], f16, addr_space="Shared")
            wh1_d = dpool.tile([WH1PAD], f32)

            with tc.tile_pool(name="const", bufs=1) as cpool:
                w_sb = cpool.tile([IN_F, OUT_F], f32)
                nc.sync.dma_start(out=w_sb[:], in_=W_in[:])
                a2_sb = cpool.tile([OUT_F, 2], f32)
                nc.sync.dma_start(out=a2_sb[:], in_=a2_in[:])
                bias_col = cpool.tile([OUT_F, 1], f32)
                nc.sync.dma_start(out=bias_col[:], in_=bias_in[:, None])
                e8_sb = cpool.tile([8, P], f32)
                nc.sync.dma_start(out=e8_sb[:], in_=e8_in[:])
                s8_sb = cpool.tile([P, 8], f32)
                nc.sync.dma_start(out=s8_sb[:], in_=s8_in[:])
                id2_sb = cpool.tile([2, 2], f32)
                nc.sync.dma_start(out=id2_sb[:], in_=id2_in[:])
                # bias broadcast to all partitions
                bias_rep = cpool.tile([P, OUT_F], f32)
                nc.sync.dma_start(
                    out=bias_rep[:],
                    in_=bass.AP(bias_in.handle if hasattr(bias_in, "handle") else bias_in, 0, [[0, P], [1, OUT_F]]),
                )

                # Wa2 = W @ [a_dst | a_src]  (contract over OUT_F): need W^T.
                with tc.tile_pool(name="pa", bufs=1, space="PSUM") as pp0, tc.tile_pool(
                    name="sa", bufs=1
                ) as sp0:
                    idp = sp0.tile([P, P], f32)
                    from concourse.masks import make_identity

                    make_identity(nc, idp[:])
                    wt_ps = pp0.tile([P, P], f32, space="PSUM")
                    nc.tensor.transpose(out=wt_ps[:OUT_F, :IN_F], in_=w_sb[:], identity=idp[:])
                    wt_sb = sp0.tile([OUT_F, IN_F], f32)
                    nc.vector.tensor_copy(out=wt_sb[:], in_=wt_ps[:OUT_F, :IN_F])
                    wa2_ps = pp0.tile([IN_F, 2], f32, space="PSUM")
                    nc.tensor.matmul(out=wa2_ps[:], lhsT=wt_sb[:], rhs=a2_sb[:])
                    wa2_sb = cpool.tile([IN_F, 2], f32)
                    nc.vector.tensor_copy(out=wa2_sb[:], in_=wa2_ps[:])
                    # ab = a2^T bias  -> [2,1]; broadcast each to 128 partitions
                    ab_ps = pp0.tile([2, 1], f32, space="PSUM")
                    nc.tensor.matmul(out=ab_ps[:], lhsT=a2_sb[:], rhs=bias_col[:])
                    ab_sb = sp0.tile([2, 1], f32)
                    nc.vector.tensor_copy(out=ab_sb[:], in_=ab_ps[:])
                    ab_dram = dpool.tile([2], f32)
                    nc.sync.dma_start(
                        out=bass.AP(ab_dram[:].tensor, ab_dram[:].offset, [[1, 2], [1, 1]]),
                        in_=ab_sb[:],
                    )
                    cv2_rep = cpool.tile([P, 1], f32)
                    cv1_rep = cpool.tile([P, 1], f32)
                    nc.sync.dma_start(
                        out=cv2_rep[:],
                        in_=bass.AP(ab_dram[:].tensor, ab_dram[:].offset, [[0, P], [1, 1]]),
                    )
                    nc.sync.dma_start(
                        out=cv1_rep[:],
                        in_=bass.AP(ab_dram[:].tensor, ab_dram[:].offset + 1, [[0, P], [1, 1]]),
                    )

                # ---------------- phase A: h / wh1 / wh2 for own rows
                with tc.tile_pool(name="pha_s", bufs=1) as spA, tc.tile_pool(
                    name="pha_ps", bufs=2, space="PSUM"
                ) as ppA, tc.tile_pool(name="pha_w", bufs=2) as wpA:
                    inT_sb = spA.tile([IN_F, NLOC], f32)
                    nc.sync.dma_start(out=inT_sb[:], in_=inT[:])
                    wh1_all = spA.tile([P, NBLK], f32)
                    nc.vector.memset(wh1_all[:], 0.0)

                    for t in range(NBLK):
                        r0 = t * P
                        rows = min(P, NLOC - r0)
                        lhsT = inT_sb[:, r0 : r0 + rows]
                        h_ps = ppA.tile([P, OUT_F], f32, tag="h_ps")
                        nc.tensor.matmul(out=h_ps[:rows, :], lhsT=lhsT, rhs=w_sb[:])
                        whT_ps = ppA.tile([P, 2], f32, tag="whT_ps")
                        nc.tensor.matmul(out=whT_ps[:rows, :], lhsT=lhsT, rhs=wa2_sb[:])

                        h16 = wpA.tile([P, NPITCH], f16, tag="h16")
                        nc.vector.tensor_add(
                            out=h16[:rows, 0:OUT_F], in0=h_ps[:rows, :], in1=bias_rep[:rows, :]
                        )
                        h16f32 = h16[:].bitcast(f32)
                        nc.vector.tensor_add(
                            out=h16f32[:rows, 32:33], in0=whT_ps[:rows, 0:1], in1=cv2_rep[:rows, :]
                        )
                        nc.vector.tensor_add(
                            out=wh1_all[:rows, t : t + 1],
                            in0=whT_ps[:rows, 1:2],
                            in1=cv1_rep[:rows, :],
                        )
                        # store 128 rows = 32 table rows at node pitch
                        nc.sync.dma_start(
                            out=bass.AP(
                                h4_loc[:].tensor,
                                h4_loc[:].offset + (r0 // PACK) * RSTRIDE,
                                [[NPITCH, rows], [1, NPITCH]],
                            ),
                            in_=h16[:rows, :],
                        )

                    # wh1_all -> wh1_d (transpose so DRAM rows are contiguous)
                    wh1T_ps = ppA.tile([NBLK, P], f32, tag="wh1T")
                    idp2 = spA.tile([P, P], f32)
                    from concourse.masks import make_identity as mkid2

                    mkid2(nc, idp2[:])
                    nc.tensor.transpose(out=wh1T_ps[:], in_=wh1_all[:], identity=idp2[:])
                    wh1T_sb = spA.tile([NBLK, P], f32)
                    nc.vector.tensor_copy(out=wh1T_sb[:], in_=wh1T_ps[:])
                    nc.sync.dma_start(
                        out=bass.AP(wh1_d[:].tensor, wh1_d[:].offset, [[P, NBLK], [1, P]]),
                        in_=wh1T_sb[:],
                    )

                # ---------------- all-gather the packed table
                nc.gpsimd.collective_compute(
                    "AllGather",
                    mybir.AluOpType.bypass,
                    replica_groups=[list(range(NCORES))],
                    ins=[h4_loc.opt()],
                    outs=[h4.opt()],
                )

                # ---------------- phase B: per-block gather + softmax + aggregate
                with tc.tile_pool(name="phb", bufs=4) as bp, tc.tile_pool(
                    name="phb_ps", bufs=1, space="PSUM"
                ) as bpp, tc.tile_pool(
                    name="phb_po", bufs=2, space="PSUM"
                ) as bpo, tc.tile_pool(name="phb_g", bufs=4) as gp:
                    s8_16 = cpool.tile([P, 8], f16)
                    nc.vector.tensor_copy(out=s8_16[:], in_=s8_sb[:])

                    for b in range(NBLK):
                        rows = P if b < NBLK - 1 else LAST_VALID
                        offs = bp.tile([P, 2, GIDX // 16], i16, tag="offs")
                        nc.sync.dma_start(
                            out=offs[:],
                            in_=idx16w[b].rearrange("g p f -> p g f"),
                        )
                        oneh = bp.tile([P, DEG, PACK], f32, tag="oneh")
                        nc.sync.dma_start(
                            out=oneh[:].rearrange("p a b -> p (a b)"), in_=oneh32[b]
                        )
                        soh = bp.tile([P, DEG, PACK, 8], f16, tag="soh")
                        nc.sync.dma_start(
                            out=soh[:].rearrange("p a b c -> p (a b c)"), in_=soh16[b]
                        )
                        wh1b = bp.tile([8, DEG], f32, tag="wh1b")
                        nc.sync.dma_start(
                            out=wh1b[:],
                            in_=bass.AP(
                                wh1_d[:].tensor,
                                wh1_d[:].offset + b * P,
                                [[1, 8], [8, DEG]],
                            ),
                        )

                        g4 = gp.tile([P, DEG, FETCH], f16, tag="g4")
                        for half in range(2):
                            nc.gpsimd.dma_gather(
                                out_ap=g4[:, half * 8 : (half + 1) * 8, :],
                                in_ap=h4[:, 0:FETCH],
                                idxs_ap=offs[:, half, :],
                                num_idxs=GIDX,
                                num_idxs_reg=GIDX,
                                elem_size=FETCH,
                                elem_step=RSTRIDE,
                                queue_num=(2 * b + half) % 4,
                            )

                        # wh2 per edge: select among the 4 packed nodes
                        g4f32 = g4[:].bitcast(f32)  # [P, DEG, FETCH//2]
                        wh2all = bass.AP(
                            g4f32.tensor,
                            g4f32.offset + 32,
                            [list(g4f32.ap[0]), [FETCH // 2, DEG], [NPITCH // 2, PACK]],
                        )
                        wh2sel = bp.tile([P, DEG, PACK], f32, tag="wh2sel")
                        nc.vector.tensor_mul(out=wh2sel[:], in0=wh2all, in1=oneh[:])
                        wh2e = bp.tile([P, DEG], f32, tag="wh2e")
                        nc.vector.reduce_sum(
                            out=wh2e[:], in_=wh2sel[:], axis=mybir.AxisListType.X
                        )

                        # e = leakyrelu(wh1_e + wh2_e)
                        wh1e_ps = bpp.tile([P, DEG], f32, space="PSUM", tag="wh1e")
                        nc.tensor.matmul(out=wh1e_ps[:], lhsT=e8_sb[:], rhs=wh1b[:])
                        epre = bp.tile([P, DEG], f32, tag="epre")
                        nc.vector.tensor_add(out=epre[:], in0=wh1e_ps[:], in1=wh2e[:])
                        esc = bp.tile([P, DEG], f32, tag="esc")
                        nc.vector.tensor_scalar_mul(out=esc[:], in0=epre[:], scalar1=ALPHA)
                        elr = bp.tile([P, DEG], f32, tag="elr")
                        nc.vector.tensor_tensor(
                            out=elr[:], in0=epre[:], in1=esc[:], op=mybir.AluOpType.max
                        )
                        ex = bp.tile([P, DEG], f32, tag="ex")
                        nc.scalar.activation(
                            out=ex[:], in_=elr[:], func=mybir.ActivationFunctionType.Exp
                        )

                        # denominators: 16-partition segment sums on PE
                        den_ps = bpp.tile([8, DEG], f32, space="PSUM", tag="den")
                        nc.tensor.matmul(out=den_ps[:], lhsT=s8_sb[:], rhs=ex[:])
                        dene = bp.tile([8, DEG], f32, tag="dene")
                        nc.vector.tensor_scalar_add(out=dene[:], in0=den_ps[:], scalar1=EPS)
                        rden = bp.tile([8, DEG], f32, tag="rden")
                        nc.vector.reciprocal(out=rden[:], in_=dene[:])
                        rdene_ps = bpp.tile([P, DEG], f32, space="PSUM", tag="rdene")
                        nc.tensor.matmul(out=rdene_ps[:], lhsT=e8_sb[:], rhs=rden[:])
                        attn = bp.tile([P, DEG], f32, tag="attn")
                        nc.vector.tensor_mul(out=attn[:], in0=ex[:], in1=rdene_ps[:])

                        # SAM = attn (bcast over m,r) * SOH -- on ScalarE with
                        # per-partition scale, per chunk (DVE step-0 bcast is slow)
                        sam = bp.tile([P, DEG, PACK, 8], f16, tag="sam")
                        for c in range(DEG):
                            nc.scalar.activation(
                                out=sam[:, c, :, :],
                                in_=soh[:, c, :, :],
                                func=mybir.ActivationFunctionType.Copy,
                                scale=attn[:, c : c + 1],
                            )

                        # aggregation: 16 chunks x 4 node-slots, PSUM-accumulated.
                        # PE outputs must start at partition 0/32/64, so chunk
                        # c's 8 dests go to psum[0:8, c, :] (dest d = 8c + d%8).
                        outp = bpo.tile([8, DEG, OUT_F], f32, space="PSUM", tag="outp")
                        for c in range(DEG):
                            for m in range(PACK):
                                nc.tensor.matmul(
                                    out=outp[:, c, :],
                                    lhsT=sam[:, c, m, :],
                                    rhs=g4[:, c, m * NPITCH : m * NPITCH + OUT_F],
                                    start=(m == 0),
                                    stop=(m == PACK - 1),
                                )
                        outsb = bp.tile([8, DEG, OUT_F], f32, tag="outsb")
                        nc.scalar.copy(
                            out=outsb[:].rearrange("p a b -> p (a b)"),
                            in_=outp[:].rearrange("p a b -> p (a b)"),
                        )
                        kfull = rows // 8  # full chunks of 8 dests
                        crem = rows - kfull * 8
                        if kfull:
                            nc.sync.dma_start(
                                out=bass.AP(
                                    out_d[:].tensor,
                                    out_d[:].offset + b * P * OUT_F,
                                    [[OUT_F, 8], [8 * OUT_F, kfull], [1, OUT_F]],
                                ),
                                in_=outsb[:, 0:kfull, :],
                            )
                        if crem:
                            nc.sync.dma_start(
                                out=bass.AP(
                                    out_d[:].tensor,
                                    out_d[:].offset + (b * P + kfull * 8) * OUT_F,
                                    [[OUT_F, crem], [8 * OUT_F, 1], [1, OUT_F]],
                                ),
                                in_=outsb[0:crem, kfull : kfull + 1, :],
                            )

    nc.compile()
    return nc


# ---------------------------------------------------------------- host side
def _host_prep(input_h, W, a, bias, indices):
    """Build the 8 per-core in_maps. Index-side layout prep only."""
    idx = np.ascontiguousarray(indices.astype(np.int32))
    in_maps = []
    # constants shared by all cores
    a2 = np.concatenate([a[OUT_F:], a[:OUT_F]], axis=1).astype(np.float32)  # [64,2] = [a_dst|a_src]
    e8 = np.zeros((8, P), dtype=np.float32)
    for pp in range(P):
        e8[pp // 16, pp] = 1.0
    s8 = np.ascontiguousarray(e8.T)
    id2 = np.eye(2, dtype=np.float32)

    for c in range(NCORES):
        r0 = c * NLOC
        inT = np.ascontiguousarray(input_h[r0 : r0 + NLOC].T)
        ecols = idx[r0 * DEG : (r0 + NLOC) * DEG]
        ecols = np.pad(ecols, (0, EPAD - ELOC))          # [EPAD]
        colq = (ecols >> 2).astype(np.int16)             # table row
        colm = (ecols & 3).astype(np.int64)              # node slot

        # int16 wrap tiles, replicated across the 8 partition groups
        cw = colq.reshape(NBLK, 2, GIDX // 16, 16)       # [b, g, i, q]
        idx16w = np.ascontiguousarray(cw.transpose(0, 1, 3, 2))  # [b, g, 16, i]
        idx16w = np.tile(idx16w, (1, 1, 8, 1))           # [b, g, 128, i]

        # edge-major one-hot of col&3: position j -> (partition j%128, chunk j//128)
        pm = colm.reshape(NBLK, DEG, P)                  # [b, chunk, part]
        oneh = np.zeros((NBLK, P, DEG, PACK), dtype=np.float32)
        bb, cc2, pp2 = np.meshgrid(
            np.arange(NBLK), np.arange(DEG), np.arange(P), indexing="ij"
        )
        oneh[bb, pp2, cc2, pm[bb, cc2, pp2]] = 1.0
        soh = (oneh[:, :, :, :, None] * e8.T.reshape(1, P, 1, 1, 8)).astype(np.float16)

        in_maps.append(
            {
                "inT": inT.astype(np.float32),
                "W_in": np.asarray(W, dtype=np.float32),
                "a2_in": a2,
                "bias_in": np.asarray(bias, dtype=np.float32),
                "idx16w": idx16w.reshape(NBLK, 2, P, GIDX // 16),
                "oneh32": oneh.reshape(NBLK, P, DEG * PACK).astype(np.float32),
                "soh16": soh.reshape(NBLK, P, DEG * PACK * 8),
                "e8_in": e8,
                "s8_in": s8,
                "id2_in": id2,
            }
        )
    return in_maps


def _reference_numpy(input_h, W, a, bias, indptr, indices):
    """Exact CPU fallback mirroring the jax reference (used only if the CSR is
    not the uniform-degree layout this kernel is specialized for)."""
    h = input_h.astype(np.float64) @ W.astype(np.float64) + bias.astype(np.float64)
    deg = np.diff(indptr.astype(np.int64))
    row = np.repeat(np.arange(N, dtype=np.int64), deg)
    e_cnt = indices.shape[0]
    if row.shape[0] < e_cnt:
        pad_val = row[-1] if row.shape[0] else 0
        row = np.pad(row, (0, e_cnt - row.shape[0]), constant_values=pad_val)
    row = row[:e_cnt]
    col = indices.astype(np.int64)
    a_src = a[:OUT_F, 0].astype(np.float64)
    a_dst = a[OUT_F:, 0].astype(np.float64)
    wh1 = h @ a_src
    wh2 = h @ a_dst
    e = wh1[row] + wh2[col]
    e = np.where(e >= 0, e, ALPHA * e)
    emax = np.full(N, -np.inf)
    np.maximum.at(emax, row, e)
    ex = np.exp(e - emax[row])
    den = np.zeros(N)
    np.add.at(den, row, ex)
    attn = ex / (den[row] + EPS)
    out = np.zeros((N, OUT_F))
    np.add.at(out, row, attn[:, None] * h[col])
    return out.astype(np.float32)


def kernel(input_h, W, a, bias, indptr, indices):
    input_h = np.asarray(input_h, dtype=np.float32)
    W = np.asarray(W, dtype=np.float32)
    a = np.asarray(a, dtype=np.float32)
    bias = np.asarray(bias, dtype=np.float32)
    indptr = np.asarray(indptr)
    indices_np = np.asarray(indices)

    expected_indptr = np.arange(N + 1, dtype=np.int64) * DEG
    if (
        indptr.shape[0] != N + 1
        or indices_np.shape[0] != E
        or not np.array_equal(indptr.astype(np.int64), expected_indptr)
    ):
        return _reference_numpy(input_h, W, a, bias, indptr, indices_np)

    _install_ntff_shim()
    _install_dma_gather_patch()
    from concourse.bass_utils import run_bass_kernel_spmd

    key = "gat"
    if key not in _PROGRAM_CACHE:
        _PROGRAM_CACHE[key] = build_program()
    nc = _PROGRAM_CACHE[key]

    in_maps = _host_prep(input_h, W, a, bias, indices_np)
    res = run_bass_kernel_spmd(nc, in_maps, core_ids=list(range(NCORES)))
    out = np.concatenate([res.results[c]["out_d"] for c in range(NCORES)], axis=0)
    return out.astype(np.float32)


if __name__ == "__main__":
    pass



# revision 32
# speedup vs baseline: 1.4131x; 1.4131x over previous
"""GAT layer kernel for Trainium2 (8 NeuronCores, Bass/Tile).

Problem: h = input_h @ W + bias; per-edge e = leakyrelu(wh1[row] + wh2[col]);
segment softmax over each destination row's 16 edges; out = segment_sum of
attn * h[col].

Strategy: destination rows sharded across 8 cores (12500 each). Each core
computes h/wh1/wh2 for its own rows into a packed fp16 table (4 nodes per
768B row, m-major: [4 x (64 h, wh2, 3 pad)]), all-gathers the table, then per
128-dest block does ONE dma_gather of 2048 edges in dest-major order
(position j = k*128 + p so dest p's 16 edges land in partitions p, slots k).
All softmax + aggregation is done with per-partition DVE/Act ops:
  - wh2[col] select (1-of-4) via one-hot multiply + reduce
  - e = leakyrelu(wh1[p] + wh2e) via fused scalar_tensor_tensor
  - ex = exp(e) on Act with fused accum_out denominator
  - select+scale of h via ONE 4x-mode scalar_tensor_tensor multiply with a
    pair-split access pattern ([p, (k,m), f/2, 2]) so the broadcast operand
    keeps a stride-1 last dim
  - segment sum via k-halving stt-add tree + m pair-adds (4x mode)
Host-side prep is limited to index layout (col>>2 wrap tiles, col&3 one-hot
masks). All numerics on h/W/a/bias happen on device.
"""

import sys
import types

import numpy as np

sys.path.insert(0, "/opt/trn_rl_repo")

# ---------------------------------------------------------------- constants
N = 100000
DEG = 16
E = N * DEG
IN_F = 128
OUT_F = 64
ALPHA = 0.2
EPS = 1e-12

NCORES = 8
NLOC = N // NCORES              # 12500 dest rows per core
P = 128
NBLK = (NLOC + P - 1) // P      # 98 blocks of 128 dests
LAST_VALID = NLOC - (NBLK - 1) * P  # 84 dests in last block

PACK = 4                        # nodes per table row
NPITCH = 66                     # fp16 elems per node slot (64 h + wh2 + 1 pad)
RSTRIDE = 384                   # fp16 elems per row (768B, must be %256B)
FETCH = PACK * NPITCH           # 272 fp16 = 544B fetched per edge
TROWS = N // PACK               # 25000 table rows
TROWS_LOC = NLOC // PACK        # 3125 local table rows
GIDX = 1024                     # indices per dma_gather (2 per block)
SPLIT_A = 48 * 32               # local table rows in allgather chunk A (of 3125)
SPLIT_B = TROWS_LOC - SPLIT_A   # rows in chunk B


def _install_ntff_shim():
    if "antenv.axon_hooks" in sys.modules:
        return
    try:
        from trn_agent_boot.trn_boot import _ntff_profile_via_ctypes

        hook = _ntff_profile_via_ctypes("/opt/axon/libaxon_pjrt.so")
    except Exception:
        hook = None
    mod = types.ModuleType("antenv.axon_hooks")
    mod.get_axon_ntff_profile_hook = lambda: hook
    mod.set_axon_ntff_profile_hook = lambda h: None
    sys.modules["antenv.axon_hooks"] = mod


def _install_dma_gather_patch():
    """Relax bass's elem_size%256 assert (ucode needs it only for transpose)."""
    import inspect
    import textwrap

    import concourse.bass as bass

    if getattr(bass.BassGpSimd.dma_gather, "_gat_patched", False):
        return
    src = textwrap.dedent(inspect.getsource(bass.BassGpSimd.dma_gather))
    old = """    assert (
        elem_size_bytes > 0 and elem_size_bytes % 256 == 0
    )  # transpose restriction"""
    new = """    assert elem_size_bytes > 0
    if transpose:
        assert elem_size_bytes % 256 == 0"""
    assert old in src, "dma_gather source changed; patch needs updating"
    src = src.replace(old, new)
    g = dict(bass.__dict__)
    exec(src, g)
    g["dma_gather"]._gat_patched = True
    bass.BassGpSimd.dma_gather = g["dma_gather"]


# ---------------------------------------------------------------- program
_PROGRAM_CACHE = {}


def build_program():
    _install_ntff_shim()
    _install_dma_gather_patch()
    import concourse.bacc as bacc
    import concourse.bass as bass
    import concourse.tile as tile
    from concourse import mybir

    f32 = mybir.dt.float32
    f16 = mybir.dt.float16
    bf16 = mybir.dt.bfloat16
    i16 = mybir.dt.int16
    Alu = mybir.AluOpType

    nc = bacc.Bacc(
        "TRN2",
        num_devices=NCORES,
        num_swdge_queues=4,
        detect_race_conditions=False,
    )

    # per-core external inputs
    inT = nc.dram_tensor("inT", [IN_F, NLOC], f32, kind="ExternalInput")
    W_in = nc.dram_tensor("W_in", [IN_F, OUT_F], f32, kind="ExternalInput")
    a2_in = nc.dram_tensor("a2_in", [OUT_F, 2], f32, kind="ExternalInput")
    bias_in = nc.dram_tensor("bias_in", [OUT_F], f32, kind="ExternalInput")
    idx16w = nc.dram_tensor("idx16w", [NBLK // 2, 4, P, GIDX // 16], i16, kind="ExternalInput")
    oneh2 = nc.dram_tensor("oneh2", [NBLK // 2, P, 2 * DEG * PACK * 2], f16, kind="ExternalInput")

    out_d = nc.dram_tensor("out_d", [NLOC, OUT_F], f32, kind="ExternalOutput")

    with tile.TileContext(nc) as tc:
        with tc.tile_pool(name="dram", bufs=1, space="DRAM") as dpool:
            h4_locA = dpool.tile([SPLIT_A, RSTRIDE], f16)
            h4_locB = dpool.tile([TROWS_LOC - SPLIT_A, RSTRIDE], f16)
            # raw (untracked) Shared table so two chunked collectives can both
            # write it; gather ordering is enforced with an explicit semaphore
            h4 = nc.dram_tensor(
                "h4tab", [TROWS, RSTRIDE], f16, kind="Internal",
                addr_space="Shared",
            )


            with tc.tile_pool(name="const", bufs=1) as cpool:
                w_sb = cpool.tile([IN_F, OUT_F], f32)
                nc.sync.dma_start(out=w_sb[:], in_=W_in[:])
                a2_sb = cpool.tile([OUT_F, 2], f32)
                nc.sync.dma_start(out=a2_sb[:], in_=a2_in[:])
                bias_col = cpool.tile([OUT_F, 1], f32)
                nc.sync.dma_start(out=bias_col[:], in_=bias_in[:, None])
                # bias broadcast to all partitions
                bias_rep = cpool.tile([P, OUT_F], f32)
                nc.sync.dma_start(
                    out=bias_rep[:],
                    in_=bass.AP(bias_in.handle if hasattr(bias_in, "handle") else bias_in, 0, [[0, P], [1, OUT_F]]),
                )

                # rhs = [W | W@a_dst | W@a_src]  (a2 = [a_dst | a_src])
                rhs_sb = cpool.tile([IN_F, OUT_F + 2], f32)
                cv2_rep = cpool.tile([P, 1], f32)  # bias . a_dst
                cv1_rep = cpool.tile([P, 1], f32)  # bias . a_src
                with tc.tile_pool(name="pa", bufs=1, space="PSUM") as pp0, tc.tile_pool(
                    name="sa", bufs=1
                ) as sp0:
                    idp = sp0.tile([P, P], f32)
                    from concourse.masks import make_identity

                    make_identity(nc, idp[:])
                    wt_ps = pp0.tile([P, P], f32, space="PSUM")
                    nc.tensor.transpose(out=wt_ps[:OUT_F, :IN_F], in_=w_sb[:], identity=idp[:])
                    wt_sb = sp0.tile([OUT_F, IN_F], f32)
                    nc.vector.tensor_copy(out=wt_sb[:], in_=wt_ps[:OUT_F, :IN_F])
                    wa2_ps = pp0.tile([IN_F, 2], f32, space="PSUM")
                    nc.tensor.matmul(out=wa2_ps[:], lhsT=wt_sb[:], rhs=a2_sb[:])
                    nc.vector.tensor_copy(out=rhs_sb[:, 0:OUT_F], in_=w_sb[:])
                    nc.vector.tensor_copy(out=rhs_sb[:, OUT_F : OUT_F + 2], in_=wa2_ps[:])
                    # ab = a2^T bias -> [2,1]; broadcast each to 128 partitions
                    ab_ps = pp0.tile([2, 1], f32, space="PSUM")
                    nc.tensor.matmul(out=ab_ps[:], lhsT=a2_sb[:], rhs=bias_col[:])
                    ab_sb = sp0.tile([2, 1], f32)
                    nc.vector.tensor_copy(out=ab_sb[:], in_=ab_ps[:])
                    ab_dram = dpool.tile([2], f32)
                    nc.sync.dma_start(
                        out=bass.AP(ab_dram[:].tensor, ab_dram[:].offset, [[1, 2], [1, 1]]),
                        in_=ab_sb[:],
                    )
                    nc.sync.dma_start(
                        out=cv2_rep[:],
                        in_=bass.AP(ab_dram[:].tensor, ab_dram[:].offset, [[0, P], [1, 1]]),
                    )
                    nc.sync.dma_start(
                        out=cv1_rep[:],
                        in_=bass.AP(ab_dram[:].tensor, ab_dram[:].offset + 1, [[0, P], [1, 1]]),
                    )

                # bf16 copy of the matmul rhs (phase-A matmuls run bf16)
                rhs16 = cpool.tile([IN_F, OUT_F + 2], bf16)
                nc.vector.tensor_copy(out=rhs16[:], in_=rhs_sb[:])

                # wh1 for all local dests, laid out [partition, block]
                wh1_all = cpool.tile([P, NBLK], f32)
                nc.vector.memset(wh1_all[:], 0.0)

                # ---------------- phase A: h / wh1 / wh2 for own rows
                with tc.tile_pool(name="pha_s", bufs=1) as spA, tc.tile_pool(
                    name="pha_ps", bufs=2, space="PSUM"
                ) as ppA, tc.tile_pool(name="pha_w", bufs=3) as wpA:
                    inT_sb = spA.tile([IN_F, NLOC], f32)
                    inT16 = spA.tile([IN_F, NLOC], bf16)
                    for ch in range(4):
                        c0 = ch * (NLOC // 4)
                        c1 = NLOC if ch == 3 else (ch + 1) * (NLOC // 4)
                        nc.sync.dma_start(
                            out=inT_sb[:, c0:c1], in_=inT[:, c0:c1]
                        )
                        nc.scalar.copy(
                            out=inT16[:, c0:c1], in_=inT_sb[:, c0:c1]
                        )

                    for j in range(NBLK // 2):
                        t0 = 2 * j
                        lastp = j == NBLK // 2 - 1
                        h_ps = ppA.tile([P, 2, OUT_F + 2], f32, tag="h_ps")
                        for s in range(2):
                            r0 = (t0 + s) * P
                            rows = min(P, NLOC - r0)
                            nc.tensor.matmul(
                                out=h_ps[:rows, s, :],
                                lhsT=inT16[:, r0 : r0 + rows],
                                rhs=rhs16[:],
                            )

                        h16 = wpA.tile([P, 2, NPITCH], f16, tag="h16")
                        nc.vector.tensor_tensor(
                            out=h16[:, :, 0:OUT_F],
                            in0=h_ps[:, :, 0:OUT_F],
                            in1=bass.AP(
                                bias_rep[:].tensor, bias_rep[:].offset,
                                [list(bias_rep[:].ap[0]), [0, 2], [1, OUT_F]],
                            ),
                            op=Alu.add,
                        )
                        # wh2 (col term) -> f16 slot 64 of both blocks
                        nc.vector.tensor_tensor(
                            out=h16[:, :, OUT_F : OUT_F + 1],
                            in0=h_ps[:, :, OUT_F : OUT_F + 1],
                            in1=bass.AP(
                                cv2_rep[:].tensor, cv2_rep[:].offset,
                                [list(cv2_rep[:].ap[0]), [0, 2], [1, 1]],
                            ),
                            op=Alu.add,
                        )
                        # wh1 (row term) -> wh1_all[:, t0:t0+2]  (garbage rows of
                        # the final partial tile stay off-path: the last store
                        # slices valid rows only)
                        nc.vector.tensor_tensor(
                            out=wh1_all[:, t0 : t0 + 2],
                            in0=h_ps[:, :, OUT_F + 1 : OUT_F + 2],
                            in1=bass.AP(
                                cv1_rep[:].tensor, cv1_rep[:].offset,
                                [list(cv1_rep[:].ap[0]), [0, 2], [1, 1]],
                            ),
                            op=Alu.add,
                        )
                        # store 256 nodes = 64 table rows (4 nodes each, m-major)
                        trow = t0 * (P // PACK)
                        slab = h4_locA if trow < SPLIT_A else h4_locB
                        soff = trow if trow < SPLIT_A else trow - SPLIT_A
                        rows1 = LAST_VALID if lastp else P
                        nc.sync.dma_start(
                            out=bass.AP(
                                slab[:].tensor,
                                slab[:].offset + soff * RSTRIDE,
                                [[RSTRIDE, P // PACK], [NPITCH, PACK], [1, NPITCH]],
                            ),
                            in_=h16[:, 0, :],
                        )
                        nc.sync.dma_start(
                            out=bass.AP(
                                slab[:].tensor,
                                slab[:].offset + (soff + P // PACK) * RSTRIDE,
                                [[RSTRIDE, rows1 // PACK], [NPITCH, PACK], [1, NPITCH]],
                            ),
                            in_=h16[:rows1, 1, :],
                        )

                # ---------------- all-gather the packed table in two chunks:
                # chunk A (each core's first 49 phase-A tiles) can start while
                # the tail of phase A still runs; host remaps gather indices
                # to this rank-major chunked layout
                ccA = nc.gpsimd.collective_compute(
                    "AllGather",
                    mybir.AluOpType.bypass,
                    replica_groups=[list(range(NCORES))],
                    ins=[h4_locA.opt()],
                    outs=[h4[0 : NCORES * SPLIT_A, :].opt()],
                )
                ccB = nc.gpsimd.collective_compute(
                    "AllGather",
                    mybir.AluOpType.bypass,
                    replica_groups=[list(range(NCORES))],
                    ins=[h4_locB.opt()],
                    outs=[h4[NCORES * SPLIT_A : TROWS, :].opt()],
                )

                # ---------------- phase B: per-block gather + softmax + aggregate
                with tc.tile_pool(name="phb", bufs=8) as bp, tc.tile_pool(
                    name="phb_g", bufs=5
                ) as gp, tc.tile_pool(name="phb_s", bufs=2) as sp, tc.tile_pool(
                    name="phb_t", bufs=2
                ) as tp:
                    for i in range(NBLK // 2):
                        b0 = 2 * i
                        last = i == NBLK // 2 - 1
                        offs = bp.tile([P, 4, GIDX // 16], i16, tag="offs")
                        nc.sync.dma_start(
                            out=offs[:],
                            in_=idx16w[i].rearrange("g p f -> p g f"),
                        )
                        oh2 = bp.tile([P, 2 * DEG * PACK * 2], f16, tag="oh2")
                        nc.sync.dma_start(out=oh2[:], in_=oneh2[i])

                        g4 = gp.tile([P, 2, DEG, FETCH], f16, tag="g4")
                        for q in range(4):
                            blk, half = q // 2, q % 2
                            gi = nc.gpsimd.dma_gather(
                                out_ap=g4[:, blk, half * 8 : (half + 1) * 8, :],
                                in_ap=h4[:, 0:FETCH],
                                idxs_ap=offs[:, q, :],
                                num_idxs=GIDX,
                                num_idxs_reg=GIDX,
                                elem_size=FETCH,
                                elem_step=RSTRIDE,
                                queue_num=q,
                            )
                            for cc in (ccA, ccB):
                                bass._add_dep_helper(
                                    gi.ins, cc.ins, sync=True,
                                    reason="gather reads untracked h4 table",
                                )

                        # ---- wh2 per edge: 1-of-4 select via one-hot
                        # wh2 candidates at g4[p, blk, k, 68m + 64] (m-major)
                        g4v = g4[:].rearrange("p a b c -> p (a b c)")
                        BLKW = DEG * FETCH  # 4352
                        wh2all = bass.AP(
                            g4v.tensor,
                            g4v.offset + OUT_F,
                            [list(g4v.ap[0]), [BLKW, 2], [FETCH, DEG], [NPITCH, PACK]],
                        )
                        oneh = bass.AP(
                            oh2[:].tensor,
                            oh2[:].offset,
                            [list(oh2[:].ap[0]), [DEG * PACK * 2, 2], [PACK * 2, DEG], [2, PACK]],
                        )
                        wh2sel = bp.tile([P, 2, DEG, PACK], f32, tag="wh2sel")
                        nc.vector.tensor_tensor(
                            out=wh2sel[:], in0=oneh, in1=wh2all, op=Alu.mult
                        )
                        wh2e = bp.tile([P, 2, DEG], f32, tag="wh2e")
                        nc.vector.reduce_sum(
                            out=wh2e[:], in_=wh2sel[:], axis=mybir.AxisListType.X
                        )
                        # epre = wh2e + wh1[p, blk]
                        epre = bp.tile([P, 2, DEG], f32, tag="epre")
                        nc.vector.tensor_tensor(
                            out=epre[:], in0=wh2e[:],
                            in1=bass.AP(
                                wh1_all[:].tensor,
                                wh1_all[:].offset + b0,
                                [list(wh1_all[:].ap[0]), [1, 2], [0, DEG]],
                            ),
                            op=Alu.add,
                        )
                        # leakyrelu: elr = max(alpha*epre, epre)
                        eprev = epre[:].rearrange("p a b -> p (a b)")
                        elr = bp.tile([P, 2 * DEG], f32, tag="elr")
                        nc.vector.scalar_tensor_tensor(
                            out=elr[:], in0=eprev, scalar=ALPHA, in1=eprev,
                            op0=Alu.mult, op1=Alu.max,
                        )
                        ex = bp.tile([P, 2, DEG], f32, tag="ex")
                        nc.scalar.activation(
                            out=ex[:].rearrange("p a b -> p (a b)"),
                            in_=elr[:],
                            func=mybir.ActivationFunctionType.Exp,
                        )
                        den = bp.tile([P, 2], f32, tag="den")
                        nc.vector.reduce_sum(
                            out=den[:], in_=ex[:], axis=mybir.AxisListType.X
                        )
                        rden = bp.tile([P, 2], f32, tag="rden")
                        nc.vector.reciprocal(out=rden[:], in_=den[:])
                        attn = bp.tile([P, 2, DEG], f16, tag="attn")
                        nc.vector.tensor_tensor(
                            out=attn[:], in0=ex[:],
                            in1=bass.AP(
                                rden[:].tensor, rden[:].offset,
                                [list(rden[:].ap[0]), [1, 2], [0, DEG]],
                            ),
                            op=Alu.mult,
                        )
                        # soh2[p, blk, km, i] = oneh2 * attn[p, blk, k]
                        soh2 = bp.tile([P, 2 * DEG * PACK * 2], f16, tag="soh2")
                        nc.vector.scalar_tensor_tensor(
                            out=soh2[:],
                            in0=oh2[:],
                            scalar=1.0,
                            in1=bass.AP(
                                attn[:].tensor,
                                attn[:].offset,
                                [list(attn[:].ap[0]), [1, 2 * DEG], [0, PACK * 2]],
                            ),
                            op0=Alu.mult,
                            op1=Alu.mult,
                        )
                        # sam4[p, bkm, f] = g4 * soh2[p, bkm] (one 2x tensor_tensor,
                        # pair-split AP keeps in1's last dim stride-1)
                        KM2 = 2 * DEG * PACK  # 128
                        sam4 = sp.tile([P, KM2, NPITCH], f16, tag="sam4")
                        nc.vector.tensor_tensor(
                            out=bass.AP(
                                sam4[:].tensor,
                                sam4[:].offset,
                                [list(sam4[:].ap[0]), [NPITCH, KM2], [2, NPITCH // 2], [1, 2]],
                            ),
                            in0=bass.AP(
                                g4v.tensor,
                                g4v.offset,
                                [list(g4v.ap[0]), [NPITCH, KM2], [2, NPITCH // 2], [1, 2]],
                            ),
                            in1=bass.AP(
                                soh2[:].tensor,
                                soh2[:].offset,
                                [list(soh2[:].ap[0]), [2, KM2], [0, NPITCH // 2], [1, 2]],
                            ),
                            op=Alu.mult,
                        )
                        # k-halving tree per block (blk dim kept)
                        s4f = sam4[:].rearrange("p a b -> p (a b)")
                        HW = DEG * FETCH // 2  # 2176
                        t8 = tp.tile([P, 2, HW], f16, tag="t8")
                        nc.vector.tensor_tensor(
                            out=t8[:],
                            in0=bass.AP(s4f.tensor, s4f.offset,
                                        [list(s4f.ap[0]), [BLKW, 2], [1, HW]]),
                            in1=bass.AP(s4f.tensor, s4f.offset + HW,
                                        [list(s4f.ap[0]), [BLKW, 2], [1, HW]]),
                            op=Alu.add,
                        )
                        t4 = tp.tile([P, 2, HW // 2], f16, tag="t4")
                        nc.vector.tensor_tensor(
                            out=t4[:], in0=t8[:, :, 0 : HW // 2],
                            in1=t8[:, :, HW // 2 : HW], op=Alu.add,
                        )
                        t2 = tp.tile([P, 2, HW // 4], f16, tag="t2")
                        nc.vector.tensor_tensor(
                            out=t2[:], in0=t4[:, :, 0 : HW // 4],
                            in1=t4[:, :, HW // 4 : HW // 2], op=Alu.add,
                        )
                        t1 = tp.tile([P, 2, FETCH], f16, tag="t1")  # [4m x 68f]
                        nc.vector.tensor_tensor(
                            out=t1[:], in0=t2[:, :, 0:FETCH],
                            in1=t2[:, :, FETCH : 2 * FETCH], op=Alu.add,
                        )
                        # m pair-adds: [4x68] -> [2x68] -> [68] (f32 out)
                        u = tp.tile([P, 2, NPITCH * 2], f16, tag="u")
                        nc.vector.tensor_tensor(
                            out=u[:], in0=t1[:, :, 0 : 2 * NPITCH],
                            in1=t1[:, :, 2 * NPITCH : 4 * NPITCH], op=Alu.add,
                        )
                        outsb = tp.tile([P, 2, NPITCH], f32, tag="outsb")
                        nc.vector.tensor_tensor(
                            out=outsb[:], in0=u[:, :, 0:NPITCH],
                            in1=u[:, :, NPITCH : 2 * NPITCH], op=Alu.add,
                        )
                        if not last:
                            nc.sync.dma_start(
                                out=bass.AP(
                                    out_d[:].tensor,
                                    out_d[:].offset + b0 * P * OUT_F,
                                    [[OUT_F, P], [P * OUT_F, 2], [1, OUT_F]],
                                ),
                                in_=outsb[:, :, 0:OUT_F],
                            )
                        else:
                            nc.sync.dma_start(
                                out=bass.AP(
                                    out_d[:].tensor,
                                    out_d[:].offset + b0 * P * OUT_F,
                                    [[OUT_F, P], [1, OUT_F]],
                                ),
                                in_=outsb[:, 0, 0:OUT_F],
                            )
                            nc.sync.dma_start(
                                out=bass.AP(
                                    out_d[:].tensor,
                                    out_d[:].offset + (b0 + 1) * P * OUT_F,
                                    [[OUT_F, LAST_VALID], [1, OUT_F]],
                                ),
                                in_=outsb[:LAST_VALID, 1, 0:OUT_F],
                            )

    nc.compile()
    return nc


# ---------------------------------------------------------------- host side
def _host_prep(input_h, W, a, bias, indices):
    """Build the 8 per-core in_maps. Index-side layout prep only."""
    idx = np.ascontiguousarray(indices.astype(np.int32))
    in_maps = []
    # a2 = [a_dst | a_src]
    a2 = np.concatenate([a[OUT_F:], a[:OUT_F]], axis=1).astype(np.float32)

    for c in range(NCORES):
        r0 = c * NLOC
        inT = np.ascontiguousarray(input_h[r0 : r0 + NLOC].T)
        ecols = idx[r0 * DEG : (r0 + NLOC) * DEG].reshape(NLOC, DEG)
        # pad dests to NBLK*P with col 0
        epad = np.zeros((NBLK * P, DEG), dtype=np.int32)
        epad[:NLOC] = ecols
        epad = epad.reshape(NBLK, P, DEG)
        colq32 = (epad >> 2).astype(np.int32)            # global table row
        # remap to the chunked allgather layout (rank-major chunks A then B)
        rank = colq32 // TROWS_LOC
        off = colq32 % TROWS_LOC
        colq = np.where(
            off < SPLIT_A,
            rank * SPLIT_A + off,
            NCORES * SPLIT_A + rank * SPLIT_B + (off - SPLIT_A),
        ).astype(np.int16)                               # table row
        colm = (epad & 3).astype(np.int64)               # node slot

        # dest-major index order: position j = k*128 + p  ->  idx[b, k, p];
        # two 1024-idx gathers per block (k 0..7, then k 8..15)
        ordered = np.ascontiguousarray(colq.transpose(0, 2, 1))  # [b, k, p]
        halves = ordered.reshape(NBLK, 2, GIDX // 16, 16)
        wrapped = halves.transpose(0, 1, 3, 2)           # [b, 2, 16, GIDX//16]
        idx16w = np.tile(wrapped, (1, 1, 8, 1))          # [b, 2, 128, GIDX//16]
        idx16w = idx16w.reshape(NBLK // 2, 4, P, GIDX // 16)

        # one-hot of col&3, duplicated for f-pair AP: [b, p, k, m, 2]
        oh = np.zeros((NBLK, P, DEG, PACK), dtype=np.float16)
        bb, pp2, kk = np.meshgrid(
            np.arange(NBLK), np.arange(P), np.arange(DEG), indexing="ij"
        )
        oh[bb, pp2, kk, colm[bb, pp2, kk]] = 1.0
        oh2 = np.repeat(oh[:, :, :, :, None], 2, axis=4)

        in_maps.append(
            {
                "inT": inT.astype(np.float32),
                "W_in": np.asarray(W, dtype=np.float32),
                "a2_in": a2,
                "bias_in": np.asarray(bias, dtype=np.float32),
                "idx16w": np.ascontiguousarray(idx16w),
                "oneh2": np.ascontiguousarray(
                    oh2.reshape(NBLK // 2, 2, P, DEG * PACK * 2)
                    .transpose(0, 2, 1, 3)
                    .reshape(NBLK // 2, P, 2 * DEG * PACK * 2)
                ),
            }
        )
    return in_maps


def _reference_numpy(input_h, W, a, bias, indptr, indices):
    """Exact CPU fallback mirroring the jax reference (used only if the CSR is
    not the uniform-degree layout this kernel is specialized for)."""
    h = input_h.astype(np.float64) @ W.astype(np.float64) + bias.astype(np.float64)
    deg = np.diff(indptr.astype(np.int64))
    row = np.repeat(np.arange(N, dtype=np.int64), deg)
    e_cnt = indices.shape[0]
    if row.shape[0] < e_cnt:
        pad_val = row[-1] if row.shape[0] else 0
        row = np.pad(row, (0, e_cnt - row.shape[0]), constant_values=pad_val)
    row = row[:e_cnt]
    col = indices.astype(np.int64)
    a_src = a[:OUT_F, 0].astype(np.float64)
    a_dst = a[OUT_F:, 0].astype(np.float64)
    wh1 = h @ a_src
    wh2 = h @ a_dst
    e = wh1[row] + wh2[col]
    e = np.where(e >= 0, e, ALPHA * e)
    emax = np.full(N, -np.inf)
    np.maximum.at(emax, row, e)
    ex = np.exp(e - emax[row])
    den = np.zeros(N)
    np.add.at(den, row, ex)
    attn = ex / (den[row] + EPS)
    out = np.zeros((N, OUT_F))
    np.add.at(out, row, attn[:, None] * h[col])
    return out.astype(np.float32)


def kernel(input_h, W, a, bias, indptr, indices):
    input_h = np.asarray(input_h, dtype=np.float32)
    W = np.asarray(W, dtype=np.float32)
    a = np.asarray(a, dtype=np.float32)
    bias = np.asarray(bias, dtype=np.float32)
    indptr = np.asarray(indptr)
    indices_np = np.asarray(indices)

    expected_indptr = np.arange(N + 1, dtype=np.int64) * DEG
    if (
        indptr.shape[0] != N + 1
        or indices_np.shape[0] != E
        or not np.array_equal(indptr.astype(np.int64), expected_indptr)
    ):
        return _reference_numpy(input_h, W, a, bias, indptr, indices_np)

    _install_ntff_shim()
    _install_dma_gather_patch()
    from concourse.bass_utils import run_bass_kernel_spmd

    key = "gat"
    if key not in _PROGRAM_CACHE:
        _PROGRAM_CACHE[key] = build_program()
    nc = _PROGRAM_CACHE[key]

    in_maps = _host_prep(input_h, W, a, bias, indices_np)
    res = run_bass_kernel_spmd(nc, in_maps, core_ids=list(range(NCORES)))
    out = np.concatenate([res.results[c]["out_d"] for c in range(NCORES)], axis=0)
    return out.astype(np.float32)


if __name__ == "__main__":
    pass
